# revision 33
# baseline (speedup 1.0000x reference)
"""Trainium2 Bass kernel for GatedGraphConv (Devign) GNN message passing.

Model (reference):
    h0 = pad(x, 256); 6 layers of: m = h @ w[l]; agg = scatter_add(m[src] -> dst);
    h = GRUCell(agg, h); then global mean pool per graph; 2-layer MLP classifier;
    sigmoid.

Distribution over 8 NeuronCores (v2):
    - Nodes are BIN-PACKED into (core, tile) slots by in-degree so every
      128-node dst tile receives <= 768 in-edges: the gather runs a uniform
      6 chunks/tile (294 chunks/core/layer vs 342 for the contiguous split),
      which is a 16% cut of the dominant cost (random-row gather of the
      all-gathered message table).
    - h lives SBUF-resident in bf16, feature-major; messages m = h @ w are
      computed per 128-node tile (PSUM f32) and staged node-major into two
      SBUF buffers (tiles 0-23, 24-48) which are stored with two large
      per-partition-contiguous DMAs into m_loc_a/m_loc_b (partition-major
      row layout), then AllGathered piece-wise so the first piece's
      collective overlaps the second piece's compute.
    - Per layer, the gather loop (indirect DMA, 128-row chunks), one-hot
      scatter matmuls, the GRU of the finished node blocks, and the NEXT
      layer's m-tiles are interleaved per 4-tile block so PE/scalar/vector
      work and the next AllGather hide under the gather DMA drain.
    - Mean-pool via one-hot matmul with host-prescaled 1/count entries,
      AllReduce of the [256,256] f32 graph sums, classifier on-device.

The graded entry point is kernel(**inputs) -> np.ndarray [256, 1] float32.
"""

import numpy as np
import ml_dtypes

import concourse.bacc as bacc
import concourse.bass as bass
import concourse.mybir as mybir
import concourse.tile as tile
from concourse.masks import make_identity

# Problem constants (hardcoded per the harness contract).
N_CORES = 8
N_NODES = 50000
N_EDGES = 300000
IN_DIM = 128
C = 256            # out_channels
G3 = 3 * C         # gru gate width
L = 6              # layers
N_GRAPHS = 256
P = 128
TPC = 49           # dst tiles per core
NLP = TPC * P      # 6272 padded local slots per core
NFULL = N_CORES * NLP
TSPA = 24          # tiles in AllGather piece A
TSPB = TPC - TSPA  # 25 tiles in piece B
ROWS_A = TSPA * P  # 3072
ROWS_B = TSPB * P  # 3200
NA = N_CORES * ROWS_A  # rows in message table A (gathered early)
NB = N_CORES * ROWS_B  # rows in message table B
BIGIDX = 0x40000000    # out-of-bounds marker for skipped gather lanes

# GRU node blocks: 12x512 + 1x128 = 6272.
BLOCKS = [(i * 512, 512) for i in range(12)] + [(12 * 512, 128)]

F32 = mybir.dt.float32
BF16 = mybir.dt.bfloat16
I32 = mybir.dt.int32
NP_BF16 = ml_dtypes.bfloat16
NP_FP8 = ml_dtypes.float8_e4m3


# --------------------------------------------------------------------------
# Host-side preprocessing
# --------------------------------------------------------------------------

def _balanced_slots(dst):
    """Assign the 50000 nodes to 8*49*128 slots so each (core, tile) bin's
    in-degree sum is as even as possible (target <= 768 = 6 chunks of 128).

    Returns slot_of_node [N_NODES] -> global slot id (core*NLP + tile*128 + row).
    """
    import heapq

    deg = np.bincount(dst, minlength=N_NODES)
    order = np.argsort(-deg, kind="stable")
    nbins = N_CORES * TPC
    heap = [(0, b) for b in range(nbins)]
    heapq.heapify(heap)
    fill = np.zeros(nbins, np.int32)
    sums = np.zeros(nbins, np.int64)
    slot_of_node = np.empty(N_NODES, np.int64)
    stash = []
    for n in order:
        while True:
            s, b = heapq.heappop(heap)
            if fill[b] < P:
                break
            # bin full; drop it permanently
        slot_of_node[n] = b * P + fill[b]
        fill[b] += 1
        sums[b] += deg[n]
        if fill[b] < P:
            heapq.heappush(heap, (s + deg[n], b))
    return slot_of_node, sums


def preprocess(x, edge_index, batch, weight, w_ih, w_hh, b_ih, b_hh, W1, b1, W2, b2):
    x = np.asarray(x, np.float32)
    edge_index = np.asarray(edge_index, np.int64)
    batch = np.asarray(batch, np.int64)
    weight = np.asarray(weight, np.float32)
    w_ih = np.asarray(w_ih, np.float32)
    w_hh = np.asarray(w_hh, np.float32)
    b_ih = np.asarray(b_ih, np.float32)
    b_hh = np.asarray(b_hh, np.float32)
    W1 = np.asarray(W1, np.float32)
    b1 = np.asarray(b1, np.float32)
    W2 = np.asarray(W2, np.float32)
    b2 = np.asarray(b2, np.float32)

    src = edge_index[0]
    dst = edge_index[1]

    slot_of_node, _ = _balanced_slots(dst)
    # slot decomposition
    s_core = slot_of_node // NLP
    s_loc = slot_of_node - s_core * NLP
    s_tile = s_loc // P
    s_row = s_loc - s_tile * P

    # Message-table row of a slot. The per-layer messages are AllGathered as
    # two separate tables: table A holds tiles 0-23 (rows p*24+t per core
    # shard), table B holds tiles 24-48 (rows p*25+(t-24)). A slot has a row
    # in exactly one table; the other gets BIGIDX (bounds-checked skip).
    in_a = s_tile < TSPA
    rowA_of_slot = np.where(
        in_a, s_core * ROWS_A + s_row * TSPA + s_tile, BIGIDX)
    rowB_of_slot = np.where(
        in_a, BIGIDX, s_core * ROWS_B + s_row * TSPB + (s_tile - TSPA))

    # Per-edge dst placement and src m-rows.
    e_core = s_core[dst]
    e_tile = s_tile[dst]
    e_row = s_row[dst]
    e_srowA = rowA_of_slot[src]
    e_srowB = rowB_of_slot[src]

    order = np.lexsort((e_row, e_tile, e_core))
    core_s = e_core[order]
    t_s = e_tile[order]
    r_s = e_row[order].astype(np.int32)
    srowA_s = e_srowA[order].astype(np.int32)
    srowB_s = e_srowB[order].astype(np.int32)

    key = core_s * TPC + t_s
    counts = np.bincount(key, minlength=N_CORES * TPC).reshape(N_CORES, TPC)
    starts = np.zeros(N_CORES * TPC, np.int64)
    np.cumsum(counts.ravel()[:-1], out=starts[1:])
    starts = starts.reshape(N_CORES, TPC)

    nchunk_t = np.maximum((counts.max(axis=0) + P - 1) // P, 1).astype(np.int64)
    koff = np.zeros(TPC + 1, np.int64)
    np.cumsum(nchunk_t, out=koff[1:])
    ktot = int(koff[-1])

    # Padding lanes: table A gathers row 0 (harmless; S-matrix zeroes them),
    # table B skips via BIGIDX.
    gidx_a = np.zeros((N_CORES, P, ktot), np.int32)
    gidx_b = np.full((N_CORES, P, ktot), BIGIDX, np.int32)
    dstrow = np.full((N_CORES, P, ktot), 255, np.uint8)
    for c in range(N_CORES):
        for t in range(TPC):
            k0 = starts[c, t]
            n = counts[c, t]
            rloc = r_s[k0 : k0 + n]
            j = np.arange(n)
            kk = j // P
            ee = j - kk * P
            gidx_a[c, ee, koff[t] + kk] = srowA_s[k0 : k0 + n]
            gidx_b[c, ee, koff[t] + kk] = srowB_s[k0 : k0 + n]
            dstrow[c, ee, koff[t] + kk] = rloc
    chunks = tuple(int(v) for v in nchunk_t)

    # Pooling inputs (slot -> graph id; -1 for empty slots).
    gcount = np.bincount(batch, minlength=N_GRAPHS).astype(np.float32)
    inv = 1.0 / np.maximum(gcount, 1.0)
    gidf = np.full((N_CORES, TPC, P), -1.0, np.float32)
    gidf[s_core, s_tile, s_row] = batch.astype(np.float32)
    gidf = gidf.astype(NP_BF16)
    invrow = np.broadcast_to(inv, (P, N_GRAPHS)).astype(NP_BF16).copy()
    iota128 = np.broadcast_to(np.arange(P, dtype=np.float32), (P, P)).astype(NP_BF16).copy()
    iotag = np.broadcast_to(np.arange(N_GRAPHS, dtype=np.float32), (P, N_GRAPHS)).astype(NP_BF16).copy()

    # x^T per core in slot order (h0 upper feature half zero, built on device).
    xT = np.zeros((N_CORES, IN_DIM, NLP), NP_BF16)
    xTsrc = x.T.astype(NP_BF16)  # [128, N_NODES]
    xT[s_core, :, s_loc] = xTsrc.T  # fancy-index: [N_NODES, 128] rows
    # note: xT[c, :, loc] assignment via advanced indexing transposes; the
    # line above sets xT[core, :, loc] = x[node] for each node.

    wz = weight.reshape(L, 2, P, C).astype(NP_BF16)
    wihT = np.ascontiguousarray(w_ih.T).reshape(2, P, G3).astype(NP_BF16)
    whhT = np.ascontiguousarray(w_hh.T).reshape(2, P, G3).astype(NP_BF16)
    bsum = b_ih + b_hh
    bias = np.stack(
        [bsum[0:128], bsum[128:256], bsum[256:384], bsum[384:512],
         b_ih[512:640], b_ih[640:768], b_hh[512:640], b_hh[640:768]], axis=1
    ).astype(np.float32)
    w1T = np.ascontiguousarray(W1.T).reshape(2, P, 128).astype(np.float32)
    w2T = np.ascontiguousarray(W2.T).astype(np.float32)
    b1c = b1.reshape(P, 1).astype(np.float32)
    b2c = b2.reshape(1, 1).astype(np.float32)

    in_maps = []
    for c in range(N_CORES):
        in_maps.append({
            "xT": xT[c],
            "gidx_a": gidx_a[c],
            "gidx_b": gidx_b[c],
            "dstrow": dstrow[c],
            "gidf": gidf[c],
            "invrow": invrow,
            "iota128": iota128,
            "iotag": iotag,
            "wz": wz,
            "wihT": wihT,
            "whhT": whhT,
            "bias": bias,
            "w1T": w1T,
            "b1": b1c,
            "w2T": w2T,
            "b2": b2c,
        })
    return in_maps, chunks


# --------------------------------------------------------------------------
# Device kernel
# --------------------------------------------------------------------------

def build_kernel(chunks, abl: str = '', reps: int = 1, mdt_name: str = 'bf16'):
    chunks_l = chunks
    koff = [0]
    for v in chunks:
        koff.append(koff[-1] + v)
    ktot = koff[-1]
    MDT = {'bf16': BF16, 'fp8': mybir.dt.float8e4}[mdt_name]
    nc = bacc.Bacc(None, num_devices=N_CORES)

    xT_in = nc.dram_tensor("xT", [IN_DIM, NLP], BF16, kind="ExternalInput")
    gidxa_in = nc.dram_tensor("gidx_a", [P, ktot], I32, kind="ExternalInput")
    gidxb_in = nc.dram_tensor("gidx_b", [P, ktot], I32, kind="ExternalInput")
    dstrow_in = nc.dram_tensor("dstrow", [P, ktot], mybir.dt.uint8,
                               kind="ExternalInput")
    gidf_in = nc.dram_tensor("gidf", [TPC, P], BF16, kind="ExternalInput")
    invrow_in = nc.dram_tensor("invrow", [P, N_GRAPHS], BF16, kind="ExternalInput")
    iota128_in = nc.dram_tensor("iota128", [P, P], BF16, kind="ExternalInput")
    iotag_in = nc.dram_tensor("iotag", [P, N_GRAPHS], BF16, kind="ExternalInput")
    wz_in = nc.dram_tensor("wz", [L, 2, P, C], BF16, kind="ExternalInput")
    wihT_in = nc.dram_tensor("wihT", [2, P, G3], BF16, kind="ExternalInput")
    whhT_in = nc.dram_tensor("whhT", [2, P, G3], BF16, kind="ExternalInput")
    bias_in = nc.dram_tensor("bias", [P, 8], F32, kind="ExternalInput")
    w1T_in = nc.dram_tensor("w1T", [2, P, 128], F32, kind="ExternalInput")
    b1_in = nc.dram_tensor("b1", [P, 1], F32, kind="ExternalInput")
    w2T_in = nc.dram_tensor("w2T", [P, 1], F32, kind="ExternalInput")
    b2_in = nc.dram_tensor("b2", [1, 1], F32, kind="ExternalInput")
    out = nc.dram_tensor("out", [1, N_GRAPHS], F32, kind="ExternalOutput")

    rg = [list(range(N_CORES))]
    NL = L * reps

    with tile.TileContext(nc) as tc:
        with (
            tc.tile_pool(name="persist", bufs=1) as pp,
            tc.tile_pool(name="msb", bufs=1) as msb_pool,
            tc.tile_pool(name="mg", bufs=8) as mg_pool,
            tc.tile_pool(name="ssb", bufs=3) as ssb_pool,
            tc.tile_pool(name="gsb", bufs=10) as gsb_pool,
            tc.tile_pool(name="ps", bufs=6, space="PSUM") as ps_pool,
            tc.tile_pool(name="pps", bufs=1, space="PSUM") as pps_pool,
            tc.tile_pool(name="dram", bufs=1, space="DRAM") as dr,
        ):
            m_loc = dr.tile([NLP, C], MDT, name="m_loc")
            m_fulls_a = [
                dr.tile([NA, C], MDT, addr_space="Shared", name=f"m_full_a{l}")
                for l in range(NL)
            ]
            m_fulls_b = [
                dr.tile([NB, C], MDT, addr_space="Shared", name=f"m_full_b{l}")
                for l in range(NL)
            ]
            sums_loc = dr.tile([C, N_GRAPHS], F32, name="sums_loc")
            sums_full = dr.tile([C, N_GRAPHS], F32, addr_space="Shared",
                                name="sums_full")

            # ---------------- persistent SBUF state ----------------
            h_t = [[None] * len(BLOCKS) for _ in range(2)]
            agg_t = [[None] * len(BLOCKS) for _ in range(2)]
            for cch in range(2):
                for b, (off, nb) in enumerate(BLOCKS):
                    h_t[cch][b] = pp.tile([P, nb], BF16, name=f"h_{cch}_{b}")
                    if cch == 0:
                        nc.sync.dma_start(
                            out=h_t[cch][b][:],
                            in_=xT_in[:, off : off + nb],
                        )
                    else:
                        nc.vector.memset(h_t[cch][b][:], 0.0)
                    agg_t[cch][b] = pp.tile([P, nb], BF16, name=f"agg_{cch}_{b}")

            w_sb = pp.tile([P, L * 2 * C], BF16, name="w_sb")
            for l in range(L):
                for cch in range(2):
                    nc.sync.dma_start(
                        out=w_sb[:, (l * 2 + cch) * C : (l * 2 + cch + 1) * C],
                        in_=wz_in[l, cch],
                    )
            wih_sb = pp.tile([P, 2 * G3], BF16, name="wih_sb")
            whh_sb = pp.tile([P, 2 * G3], BF16, name="whh_sb")
            for cch in range(2):
                nc.sync.dma_start(out=wih_sb[:, cch * G3 : (cch + 1) * G3],
                                  in_=wihT_in[cch])
                nc.sync.dma_start(out=whh_sb[:, cch * G3 : (cch + 1) * G3],
                                  in_=whhT_in[cch])
            bias_sb = pp.tile([P, 8], F32, name="bias_sb")
            nc.sync.dma_start(out=bias_sb[:], in_=bias_in[:])
            w1_sb = pp.tile([P, 2 * 128], F32, name="w1_sb")
            for cch in range(2):
                nc.sync.dma_start(out=w1_sb[:, cch * 128 : (cch + 1) * 128],
                                  in_=w1T_in[cch])
            b1_sb = pp.tile([P, 1], F32, name="b1_sb")
            nc.sync.dma_start(out=b1_sb[:], in_=b1_in[:])
            w2_sb = pp.tile([P, 1], F32, name="w2_sb")
            nc.sync.dma_start(out=w2_sb[:], in_=w2T_in[:])
            b2_sb = pp.tile([1, 1], F32, name="b2_sb")
            nc.sync.dma_start(out=b2_sb[:], in_=b2_in[:])

            zero_sb = pp.tile([P, max(chunks_l) * C], BF16, name="zero_sb")
            nc.vector.memset(zero_sb[:], 0.0)
            gidxa_sb = pp.tile([P, ktot], I32, name="gidxa_sb")
            nc.sync.dma_start(out=gidxa_sb[:], in_=gidxa_in[:])
            gidxb_sb = pp.tile([P, ktot], I32, name="gidxb_sb")
            nc.sync.dma_start(out=gidxb_sb[:], in_=gidxb_in[:])
            ident_sb = pp.tile([P, P], BF16, name="ident_sb")
            make_identity(nc, ident_sb[:])

            dstu8_sb = pp.tile([P, ktot], mybir.dt.uint8, name="dstu8_sb")
            nc.sync.dma_start(out=dstu8_sb[:], in_=dstrow_in[:])
            dstf_sb = pp.tile([P, ktot], BF16, name="dstf_sb")
            nc.vector.tensor_copy(dstf_sb[:], dstu8_sb[:])
            s_all = pp.tile([P, ktot * P], mybir.dt.float8e4, name="s_all")
            gidf_sb = pp.tile([P, TPC], BF16, name="gidf_sb")
            nc.sync.dma_start(
                out=gidf_sb[:],
                in_=gidf_in.rearrange("t p -> p t"),
            )
            invrow_sb = pp.tile([P, N_GRAPHS], BF16, name="invrow_sb")
            nc.sync.dma_start(out=invrow_sb[:], in_=invrow_in[:])
            iota128_sb = pp.tile([P, P], BF16, name="iota128_sb")
            nc.sync.dma_start(out=iota128_sb[:], in_=iota128_in[:])
            iotag_sb = pp.tile([P, N_GRAPHS], BF16, name="iotag_sb")
            nc.sync.dma_start(out=iotag_sb[:], in_=iotag_in[:])

            # node-major message staging for the two AllGather pieces
            m_blk_a = msb_pool.tile([P, TSPA * C], MDT, name="m_blk_a")
            m_blk_b = msb_pool.tile([P, TSPB * C], MDT, name="m_blk_b")

            def wih(cch, gc):
                return wih_sb[:, cch * G3 + gc * P : cch * G3 + (gc + 1) * P]

            def whh(cch, gc):
                return whh_sb[:, cch * G3 + gc * P : cch * G3 + (gc + 1) * P]

            def h_slice(cch, t):
                b = t // 4
                o = (t % 4) * P
                return h_t[cch][b][:, o : o + P]

            def a_tile(l, t):
                """Compute m tile t for layer l into the staging buffer."""
                psm = ps_pool.tile([P, C], F32, name="psm", tag="ps")
                for cch in range(2):
                    nc.tensor.matmul(
                        psm[:],
                        lhsT=h_slice(cch, t),
                        rhs=w_sb[:, (l * 2 + cch) * C : (l * 2 + cch + 1) * C],
                        start=(cch == 0),
                        stop=(cch == 1),
                    )
                if t < TSPA:
                    nc.scalar.copy(m_blk_a[:, t * C : (t + 1) * C], psm[:])
                else:
                    nc.scalar.copy(m_blk_b[:, (t - TSPA) * C : (t - TSPA + 1) * C],
                                   psm[:])

            def store_piece(piece):
                if piece == 0:
                    nc.sync.dma_start(
                        out=m_loc[0:ROWS_A, :].rearrange("(p t) f -> p (t f)", p=P),
                        in_=m_blk_a[:],
                    )
                else:
                    nc.sync.dma_start(
                        out=m_loc[ROWS_A:NLP, :].rearrange("(p t) f -> p (t f)", p=P),
                        in_=m_blk_b[:],
                    )

            def allgather(ll, piece):
                if "cc0" in abl:
                    return
                if "cc1" in abl and ll > 0:
                    return
                li = 0 if "cc1" in abl else ll
                if piece == 0:
                    nc.gpsimd.collective_compute(
                        "AllGather",
                        mybir.AluOpType.bypass,
                        replica_groups=rg,
                        ins=[m_loc[0:ROWS_A, :]],
                        outs=[m_fulls_a[li][:]],
                    )
                else:
                    nc.gpsimd.collective_compute(
                        "AllGather",
                        mybir.AluOpType.bypass,
                        replica_groups=rg,
                        ins=[m_loc[ROWS_A:NLP, :]],
                        outs=[m_fulls_b[li][:]],
                    )

            def gru_block(b):
                off, nb = BLOCKS[b]
                z_keep = [None, None]
                n_keep = [None, None]
                for gh in range(2):
                    psr = ps_pool.tile([P, nb], F32, name="psr", tag="ps")
                    nc.tensor.matmul(psr[:], lhsT=wih(0, gh), rhs=agg_t[0][b][:],
                                     start=True, stop=False)
                    nc.tensor.matmul(psr[:], lhsT=wih(1, gh), rhs=agg_t[1][b][:],
                                     start=False, stop=False)
                    nc.tensor.matmul(psr[:], lhsT=whh(0, gh), rhs=h_t[0][b][:],
                                     start=False, stop=False)
                    nc.tensor.matmul(psr[:], lhsT=whh(1, gh), rhs=h_t[1][b][:],
                                     start=False, stop=True)
                    r_sb = gsb_pool.tile([P, nb], BF16, name="r_sb", tag="gate")
                    nc.scalar.activation(
                        r_sb[:], psr[:], mybir.ActivationFunctionType.Sigmoid,
                        bias=bias_sb[:, gh : gh + 1],
                    )
                    psz = ps_pool.tile([P, nb], F32, name="psz", tag="ps")
                    nc.tensor.matmul(psz[:], lhsT=wih(0, 2 + gh),
                                     rhs=agg_t[0][b][:], start=True, stop=False)
                    nc.tensor.matmul(psz[:], lhsT=wih(1, 2 + gh),
                                     rhs=agg_t[1][b][:], start=False, stop=False)
                    nc.tensor.matmul(psz[:], lhsT=whh(0, 2 + gh),
                                     rhs=h_t[0][b][:], start=False, stop=False)
                    nc.tensor.matmul(psz[:], lhsT=whh(1, 2 + gh),
                                     rhs=h_t[1][b][:], start=False, stop=True)
                    z_sb = gsb_pool.tile([P, nb], BF16, name="z_sb", tag="gate")
                    nc.scalar.activation(
                        z_sb[:], psz[:], mybir.ActivationFunctionType.Sigmoid,
                        bias=bias_sb[:, 2 + gh : 3 + gh],
                    )
                    psi = ps_pool.tile([P, nb], F32, name="psi", tag="ps")
                    nc.tensor.matmul(psi[:], lhsT=wih(0, 4 + gh),
                                     rhs=agg_t[0][b][:], start=True, stop=False)
                    nc.tensor.matmul(psi[:], lhsT=wih(1, 4 + gh),
                                     rhs=agg_t[1][b][:], start=False, stop=True)
                    psh = ps_pool.tile([P, nb], F32, name="psh", tag="ps")
                    nc.tensor.matmul(psh[:], lhsT=whh(0, 4 + gh),
                                     rhs=h_t[0][b][:], start=True, stop=False)
                    nc.tensor.matmul(psh[:], lhsT=whh(1, 4 + gh),
                                     rhs=h_t[1][b][:], start=False, stop=True)
                    hn_sb = gsb_pool.tile([P, nb], BF16, name="hn_sb", tag="gate")
                    nc.scalar.activation(
                        hn_sb[:], psh[:], mybir.ActivationFunctionType.Identity,
                        bias=bias_sb[:, 6 + gh : 7 + gh],
                    )
                    rn_sb = gsb_pool.tile([P, nb], BF16, name="rn_sb", tag="gate")
                    nc.vector.tensor_mul(rn_sb[:], r_sb[:], hn_sb[:])
                    tn_sb = gsb_pool.tile([P, nb], F32, name="tn_sb", tag="gate")
                    nc.vector.tensor_add(tn_sb[:], psi[:], rn_sb[:])
                    n_sb = gsb_pool.tile([P, nb], BF16, name="n_sb", tag="gate")
                    nc.scalar.activation(
                        n_sb[:], tn_sb[:], mybir.ActivationFunctionType.Tanh,
                        bias=bias_sb[:, 4 + gh : 5 + gh],
                    )
                    z_keep[gh] = z_sb
                    n_keep[gh] = n_sb
                for gh in range(2):
                    d_sb = gsb_pool.tile([P, nb], BF16, name="d_sb", tag="gate")
                    nc.vector.tensor_sub(d_sb[:], h_t[gh][b][:], n_keep[gh][:])
                    zd_sb = gsb_pool.tile([P, nb], BF16, name="zd_sb", tag="gate")
                    nc.vector.tensor_mul(zd_sb[:], z_keep[gh][:], d_sb[:])
                    nc.vector.tensor_add(h_t[gh][b][:], n_keep[gh][:], zd_sb[:])

            # Mean-pool accumulators + per-tile pooling work; interleaved
            # into the LAST layer's per-block slot (where earlier layers
            # compute the next layer's m tiles) so it hides under the final
            # gather drain.
            pp0 = pps_pool.tile([P, N_GRAPHS], F32, name="pp0")
            pp1 = pps_pool.tile([P, N_GRAPHS], F32, name="pp1")
            ppx = [pp0, pp1]

            def pool_tile(t):
                h_rm = ssb_pool.tile([P, C], BF16, name="h_rm")
                for cch in range(2):
                    ptr = ps_pool.tile([P, P], BF16, name="ptr", tag="ps")
                    nc.tensor.transpose(ptr[:], h_slice(cch, t), ident_sb[:])
                    nc.vector.tensor_copy(h_rm[:, cch * P : (cch + 1) * P], ptr[:])
                b_sb = ssb_pool.tile([P, N_GRAPHS], BF16, name="b_sb")
                nc.vector.tensor_tensor(
                    out=b_sb[:],
                    in0=iotag_sb[:],
                    in1=gidf_sb[:, t : t + 1].to_broadcast([P, N_GRAPHS]),
                    op=mybir.AluOpType.is_equal,
                )
                nc.vector.tensor_mul(b_sb[:], b_sb[:], invrow_sb[:])
                for fh in range(2):
                    nc.tensor.matmul(
                        ppx[fh][:],
                        lhsT=h_rm[:, fh * P : (fh + 1) * P],
                        rhs=b_sb[:],
                        start=(t == 0),
                        stop=(t == TPC - 1),
                    )

            # ---------------- layer 0 prologue: m(0) and its AllGather ------
            for t in range(TPC):
                a_tile(0, t)
                if t == TSPA - 1:
                    store_piece(0)
                    allgather(0, 0)
            store_piece(1)
            allgather(0, 1)

            # ---------------- layers ----------------
            DLEAD = 0  # A-gather lead (0: HW A/B showed a lead delays the critical B-stream) (tiles): drains table-A chunks while
                       # the table-B AllGather is still in flight

            for ll in range(NL):
                l = ll % L
                li = 0 if "cc1" in abl else ll
                m_full_a = m_fulls_a[li]
                m_full_b = m_fulls_b[li]

                def issue_a(t):
                    """Memset + issue the table-A gathers for dst tile t.

                    Table-A gathers only wait on m_full_a (AllGathered mid
                    previous layer), so with a DLEAD-tile lead they drain
                    while this layer's table-B AllGather completes."""
                    ks, ke = koff[t], koff[t + 1]
                    nk = ke - ks
                    m_gA = mg_pool.tile([P, nk * C], MDT, name="m_gA", tag="mga", bufs=8)
                    nc.scalar.copy(m_gA[:], zero_sb[:, : nk * C])
                    ng = 0 if "g0" in abl else (1 if "g1" in abl else nk)
                    for k in range(ng):
                        nc.gpsimd.indirect_dma_start(
                            out=m_gA[:, k * C : (k + 1) * C],
                            out_offset=None,
                            in_=m_full_a[:],
                            in_offset=bass.IndirectOffsetOnAxis(
                                ap=gidxa_sb[:, ks + k : ks + k + 1],
                                axis=0,
                            ),
                            bounds_check=NA - 1,
                            oob_is_err=False,
                        )
                    return m_gA

                ag_a = {}
                for t in range(min(DLEAD, TPC)):
                    ag_a[t] = issue_a(t)

                # Phase C interleaved with GRU and next layer's m tiles.
                for t in range(TPC):
                    ks, ke = koff[t], koff[t + 1]
                    nk = ke - ks
                    if t + DLEAD < TPC:
                        ag_a[t + DLEAD] = issue_a(t + DLEAD)
                    m_gA = ag_a.pop(t)
                    m_gB = mg_pool.tile([P, nk * C], MDT, name="m_gB", tag="mgb", bufs=4)
                    nc.scalar.copy(m_gB[:], zero_sb[:, : nk * C])
                    ng = 0 if "g0" in abl else (1 if "g1" in abl else nk)
                    for k in range(ng):
                        nc.gpsimd.indirect_dma_start(
                            out=m_gB[:, k * C : (k + 1) * C],
                            out_offset=None,
                            in_=m_full_b[:],
                            in_offset=bass.IndirectOffsetOnAxis(
                                ap=gidxb_sb[:, ks + k : ks + k + 1],
                                axis=0,
                            ),
                            bounds_check=NB - 1,
                            oob_is_err=False,
                        )
                    if ll == 0:
                        for k in range(nk):
                            nc.vector.tensor_tensor(
                                out=s_all[:, (ks + k) * P : (ks + k + 1) * P],
                                in0=iota128_sb[:],
                                in1=dstf_sb[
                                    :, ks + k : ks + k + 1
                                ].to_broadcast([P, P]),
                                op=mybir.AluOpType.is_equal,
                            )
                    bq = t // 4
                    oq = (t % 4) * P
                    for fh in range(2):
                        psa = ps_pool.tile([P, P], F32, name="psa", tag="ps")
                        for k in range(nk):
                            nc.tensor.matmul(
                                psa[:],
                                lhsT=m_gA[:, k * C + fh * P : k * C + fh * P + P],
                                rhs=s_all[:, (ks + k) * P : (ks + k + 1) * P],
                                start=(k == 0),
                                stop=False,
                            )
                            nc.tensor.matmul(
                                psa[:],
                                lhsT=m_gB[:, k * C + fh * P : k * C + fh * P + P],
                                rhs=s_all[:, (ks + k) * P : (ks + k + 1) * P],
                                start=False,
                                stop=(k == nk - 1),
                            )
                        nc.vector.tensor_copy(
                            agg_t[fh][bq][:, oq : oq + P], psa[:]
                        )
                    # When a 4-tile block completes: GRU it, then compute the
                    # next layer's m tiles for it (overlaps the gather drain).
                    last_of_block = (t % 4 == 3) or (t == TPC - 1)
                    if last_of_block and "nogru" not in abl:
                        b = t // 4
                        gru_block(b)
                        if ll < NL - 1:
                            for tq in range(b * 4, min(b * 4 + 4, TPC)):
                                a_tile((ll + 1) % L, tq)
                            if t == TSPA - 1:
                                store_piece(0)
                            elif t == 31:
                                # traced 8 tiles after the piece-A store so the
                                # Pool queue reaches this trigger with its wait
                                # already satisfied (no gather bubble)
                                allgather(ll + 1, 0)
                            elif t == TPC - 1:
                                store_piece(1)
                                allgather(ll + 1, 1)
                        else:
                            for tq in range(b * 4, min(b * 4 + 4, TPC)):
                                pool_tile(tq)

            # ---------------- pooling tail ----------------
            if "nogru" in abl:
                for t in range(TPC):
                    pool_tile(t)
            sums_sb = pp.tile([P, 2 * N_GRAPHS], F32, name="sums_sb")
            nc.scalar.copy(sums_sb[:, 0:N_GRAPHS], pp0[:])
            nc.scalar.copy(sums_sb[:, N_GRAPHS : 2 * N_GRAPHS], pp1[:])
            nc.sync.dma_start(out=sums_loc[0:P, :], in_=sums_sb[:, 0:N_GRAPHS])
            nc.sync.dma_start(out=sums_loc[P : 2 * P, :],
                              in_=sums_sb[:, N_GRAPHS : 2 * N_GRAPHS])

            nc.gpsimd.collective_compute(
                "AllReduce",
                mybir.AluOpType.add,
                replica_groups=rg,
                ins=[sums_loc[:]],
                outs=[sums_full[:]],
            )

            # ---------------- classifier ----------------
            mt_sb = pp.tile([P, 2 * N_GRAPHS], F32, name="mt_sb")
            nc.sync.dma_start(out=mt_sb[:, 0:N_GRAPHS], in_=sums_full[0:P, :])
            nc.sync.dma_start(out=mt_sb[:, N_GRAPHS : 2 * N_GRAPHS],
                              in_=sums_full[P : 2 * P, :])
            ps1 = ps_pool.tile([P, N_GRAPHS], F32, name="ps1", tag="ps")
            for cch in range(2):
                nc.tensor.matmul(
                    ps1[:],
                    lhsT=w1_sb[:, cch * 128 : (cch + 1) * 128],
                    rhs=mt_sb[:, cch * N_GRAPHS : (cch + 1) * N_GRAPHS],
                    start=(cch == 0),
                    stop=(cch == 1),
                )
            h1_sb = pp.tile([P, N_GRAPHS], F32, name="h1_sb")
            nc.scalar.activation(
                h1_sb[:], ps1[:], mybir.ActivationFunctionType.Relu,
                bias=b1_sb[:, 0:1],
            )
            ps2 = ps_pool.tile([1, N_GRAPHS], F32, name="ps2", tag="ps")
            nc.tensor.matmul(ps2[:], lhsT=w2_sb[:, 0:1], rhs=h1_sb[:],
                             start=True, stop=True)
            out_sb = pp.tile([1, N_GRAPHS], F32, name="out_sb")
            nc.scalar.activation(
                out_sb[:], ps2[:], mybir.ActivationFunctionType.Sigmoid,
                bias=b2_sb[:, 0:1],
            )
            nc.sync.dma_start(out=out[:], in_=out_sb[:])

    nc.finalize()
    return nc


# --------------------------------------------------------------------------
# PJRT SPMD runner
# --------------------------------------------------------------------------

class SpmdRunner:
    def __init__(self, nc, n_cores):
        import jax
        from jax.experimental.shard_map import shard_map
        from jax.sharding import Mesh, PartitionSpec
        from concourse.bass2jax import (
            _bass_exec_p,
            install_neuronx_cc_hook,
            partition_id_tensor,
        )

        install_neuronx_cc_hook()
        self.jax = jax
        self.nc = nc
        self.n_cores = n_cores

        partition_name = nc.partition_id_tensor.name if nc.partition_id_tensor else None
        in_names, out_names, out_avals, zero_outs = [], [], [], []
        for alloc in nc.m.functions[0].allocations:
            if not isinstance(alloc, mybir.MemoryLocationSet):
                continue
            if not alloc.memorylocations:
                continue
            name = alloc.memorylocations[0].name
            if alloc.kind == "ExternalInput":
                if name != partition_name:
                    in_names.append(name)
            elif alloc.kind == "ExternalOutput":
                shape = tuple(alloc.tensor_shape)
                dtype = mybir.dt.np(alloc.dtype)
                out_names.append(name)
                out_avals.append(jax.core.ShapedArray(shape, dtype))
                zero_outs.append(np.zeros(shape, dtype))
        self.in_names = in_names
        self.out_names = out_names
        self.out_avals = out_avals
        self.zero_outs = zero_outs
        n_params = len(in_names)
        n_outs = len(out_names)
        all_in_names = list(in_names) + list(out_names)
        if partition_name is not None:
            all_in_names.append(partition_name)

        def _body(*args):
            operands = list(args)
            if partition_name is not None:
                operands.append(partition_id_tensor())
            outs = _bass_exec_p.bind(
                *operands,
                out_avals=tuple(out_avals),
                in_names=tuple(all_in_names),
                out_names=tuple(out_names),
                lowering_input_output_aliases=(),
                sim_require_finite=True,
                sim_require_nnan=True,
                nc=nc,
            )
            return tuple(outs)

        devices = jax.devices()[:n_cores]
        assert len(devices) == n_cores, (
            f"need {n_cores} neuron cores, found {len(jax.devices())}"
        )
        mesh = Mesh(np.asarray(devices), ("core",))
        in_specs = (PartitionSpec("core"),) * (n_params + n_outs)
        out_specs = (PartitionSpec("core"),) * n_outs
        self.fn = jax.jit(
            shard_map(_body, mesh=mesh, in_specs=in_specs, out_specs=out_specs,
                      check_rep=False),
            keep_unused=True,
        )

    def prepare(self, in_maps):
        n = self.n_cores
        concat_in = [
            np.concatenate([np.asarray(in_maps[c][name]) for c in range(n)], axis=0)
            for name in self.in_names
        ]
        concat_zero = [
            np.zeros((n * z.shape[0], *z.shape[1:]), z.dtype) for z in self.zero_outs
        ]
        return [self.jax.device_put(a) for a in concat_in + concat_zero]

    def run(self, dev_args):
        outs = self.fn(*dev_args)
        self.jax.block_until_ready(outs)
        return outs

    def results(self, outs):
        n = self.n_cores
        return [
            {
                name: np.asarray(outs[i]).reshape(n, *self.out_avals[i].shape)[c]
                for i, name in enumerate(self.out_names)
            }
            for c in range(n)
        ]


_RUNNER_CACHE = {}


def get_runner(chunks) -> SpmdRunner:
    if chunks not in _RUNNER_CACHE:
        _RUNNER_CACHE[chunks] = SpmdRunner(build_kernel(chunks), N_CORES)
    return _RUNNER_CACHE[chunks]


def kernel(**inputs) -> np.ndarray:
    in_maps, chunks = preprocess(**inputs)
    runner = get_runner(chunks)
    dev_args = runner.prepare(in_maps)
    outs = runner.run(dev_args)
    res = runner.results(outs)
    probs = np.asarray(res[0]["out"], np.float32).reshape(1, N_GRAPHS)
    return probs.T.copy()


# revision 36
# speedup vs baseline: 1.0939x; 1.0939x over previous
"""Trainium2 Bass kernel for GatedGraphConv (Devign) GNN message passing.

Model (reference):
    h0 = pad(x, 256); 6 layers of: m = h @ w[l]; agg = scatter_add(m[src] -> dst);
    h = GRUCell(agg, h); then global mean pool per graph; 2-layer MLP classifier;
    sigmoid.

Distribution over 8 NeuronCores (v2):
    - Nodes are BIN-PACKED into (core, tile) slots by in-degree so every
      128-node dst tile receives <= 768 in-edges: the gather runs a uniform
      6 chunks/tile (294 chunks/core/layer vs 342 for the contiguous split),
      which is a 16% cut of the dominant cost (random-row gather of the
      all-gathered message table).
    - h lives SBUF-resident in bf16, feature-major; messages m = h @ w are
      computed per 128-node tile (PSUM f32) and staged node-major into two
      SBUF buffers (tiles 0-23, 24-48) which are stored with two large
      per-partition-contiguous DMAs into m_loc_a/m_loc_b (partition-major
      row layout), then AllGathered piece-wise so the first piece's
      collective overlaps the second piece's compute.
    - Per layer, the gather loop (indirect DMA, 128-row chunks), one-hot
      scatter matmuls, the GRU of the finished node blocks, and the NEXT
      layer's m-tiles are interleaved per 4-tile block so PE/scalar/vector
      work and the next AllGather hide under the gather DMA drain.
    - Mean-pool via one-hot matmul with host-prescaled 1/count entries,
      AllReduce of the [256,256] f32 graph sums, classifier on-device.

The graded entry point is kernel(**inputs) -> np.ndarray [256, 1] float32.
"""

import numpy as np
import ml_dtypes

import concourse.bacc as bacc
import concourse.bass as bass
import concourse.mybir as mybir
import concourse.tile as tile
from concourse.masks import make_identity

# Problem constants (hardcoded per the harness contract).
N_CORES = 8
N_NODES = 50000
N_EDGES = 300000
IN_DIM = 128
C = 256            # out_channels
G3 = 3 * C         # gru gate width
L = 6              # layers
N_GRAPHS = 256
P = 128
TPC = 49           # dst tiles per core
NLP = TPC * P      # 6272 padded local slots per core
NFULL = N_CORES * NLP
TSPA = 24          # tiles in AllGather piece A
TSPB = TPC - TSPA  # 25 tiles in piece B
ROWS_A = TSPA * P  # 3072
ROWS_B = TSPB * P  # 3200
NA = N_CORES * ROWS_A  # rows in message table A (gathered early)
NB = N_CORES * ROWS_B  # rows in message table B
BIGIDX = 0x40000000    # out-of-bounds marker for skipped gather lanes

# GRU node blocks: 12x512 + 1x128 = 6272.
BLOCKS = [(i * 512, 512) for i in range(12)] + [(12 * 512, 128)]

F32 = mybir.dt.float32
BF16 = mybir.dt.bfloat16
I32 = mybir.dt.int32
NP_BF16 = ml_dtypes.bfloat16
NP_FP8 = ml_dtypes.float8_e4m3


# --------------------------------------------------------------------------
# Host-side preprocessing
# --------------------------------------------------------------------------

def _balanced_slots(dst):
    """Assign the 50000 nodes to 8*49*128 slots so each (core, tile) bin's
    in-degree sum is as even as possible (target <= 768 = 6 chunks of 128).

    Returns slot_of_node [N_NODES] -> global slot id (core*NLP + tile*128 + row).
    """
    import heapq

    deg = np.bincount(dst, minlength=N_NODES)
    order = np.argsort(-deg, kind="stable")
    nbins = N_CORES * TPC
    heap = [(0, b) for b in range(nbins)]
    heapq.heapify(heap)
    fill = np.zeros(nbins, np.int32)
    sums = np.zeros(nbins, np.int64)
    slot_of_node = np.empty(N_NODES, np.int64)
    stash = []
    for n in order:
        while True:
            s, b = heapq.heappop(heap)
            if fill[b] < P:
                break
            # bin full; drop it permanently
        slot_of_node[n] = b * P + fill[b]
        fill[b] += 1
        sums[b] += deg[n]
        if fill[b] < P:
            heapq.heappush(heap, (s + deg[n], b))
    return slot_of_node, sums


def preprocess(x, edge_index, batch, weight, w_ih, w_hh, b_ih, b_hh, W1, b1, W2, b2):
    x = np.asarray(x, np.float32)
    edge_index = np.asarray(edge_index, np.int64)
    batch = np.asarray(batch, np.int64)
    weight = np.asarray(weight, np.float32)
    w_ih = np.asarray(w_ih, np.float32)
    w_hh = np.asarray(w_hh, np.float32)
    b_ih = np.asarray(b_ih, np.float32)
    b_hh = np.asarray(b_hh, np.float32)
    W1 = np.asarray(W1, np.float32)
    b1 = np.asarray(b1, np.float32)
    W2 = np.asarray(W2, np.float32)
    b2 = np.asarray(b2, np.float32)

    src = edge_index[0]
    dst = edge_index[1]

    slot_of_node, _ = _balanced_slots(dst)
    # slot decomposition
    s_core = slot_of_node // NLP
    s_loc = slot_of_node - s_core * NLP
    s_tile = s_loc // P
    s_row = s_loc - s_tile * P

    # Message-table row of a slot. The per-layer messages are AllGathered as
    # two separate tables: table A holds tiles 0-23 (rows p*24+t per core
    # shard), table B holds tiles 24-48 (rows p*25+(t-24)). A slot has a row
    # in exactly one table; the other gets BIGIDX (bounds-checked skip).
    in_a = s_tile < TSPA
    rowA_of_slot = np.where(
        in_a, s_core * ROWS_A + s_row * TSPA + s_tile, BIGIDX)
    rowB_of_slot = np.where(
        in_a, BIGIDX, s_core * ROWS_B + s_row * TSPB + (s_tile - TSPA))

    # Per-edge dst placement and src m-rows.
    e_core = s_core[dst]
    e_tile = s_tile[dst]
    e_row = s_row[dst]
    e_srowA = rowA_of_slot[src]
    e_srowB = rowB_of_slot[src]

    order = np.lexsort((e_row, e_tile, e_core))
    core_s = e_core[order]
    t_s = e_tile[order]
    r_s = e_row[order].astype(np.int32)
    srowA_s = e_srowA[order].astype(np.int32)
    srowB_s = e_srowB[order].astype(np.int32)

    key = core_s * TPC + t_s
    counts = np.bincount(key, minlength=N_CORES * TPC).reshape(N_CORES, TPC)
    starts = np.zeros(N_CORES * TPC, np.int64)
    np.cumsum(counts.ravel()[:-1], out=starts[1:])
    starts = starts.reshape(N_CORES, TPC)

    nchunk_t = np.maximum((counts.max(axis=0) + P - 1) // P, 1).astype(np.int64)
    koff = np.zeros(TPC + 1, np.int64)
    np.cumsum(nchunk_t, out=koff[1:])
    ktot = int(koff[-1])

    # Padding lanes: table A gathers row 0 (harmless; S-matrix zeroes them),
    # table B skips via BIGIDX.
    gidx_a = np.zeros((N_CORES, P, ktot), np.int32)
    gidx_b = np.full((N_CORES, P, ktot), BIGIDX, np.int32)
    dstrow = np.full((N_CORES, P, ktot), 255, np.uint8)
    for c in range(N_CORES):
        for t in range(TPC):
            k0 = starts[c, t]
            n = counts[c, t]
            rloc = r_s[k0 : k0 + n]
            j = np.arange(n)
            kk = j // P
            ee = j - kk * P
            gidx_a[c, ee, koff[t] + kk] = srowA_s[k0 : k0 + n]
            gidx_b[c, ee, koff[t] + kk] = srowB_s[k0 : k0 + n]
            dstrow[c, ee, koff[t] + kk] = rloc
    chunks = tuple(int(v) for v in nchunk_t)

    # Pooling inputs (slot -> graph id; -1 for empty slots).
    gcount = np.bincount(batch, minlength=N_GRAPHS).astype(np.float32)
    inv = 1.0 / np.maximum(gcount, 1.0)
    gidf = np.full((N_CORES, TPC, P), -1.0, np.float32)
    gidf[s_core, s_tile, s_row] = batch.astype(np.float32)
    gidf = gidf.astype(NP_BF16)
    invrow = np.broadcast_to(inv, (P, N_GRAPHS)).astype(NP_BF16).copy()
    iota128 = np.broadcast_to(np.arange(P, dtype=np.float32), (P, P)).astype(NP_BF16).copy()
    iotag = np.broadcast_to(np.arange(N_GRAPHS, dtype=np.float32), (P, N_GRAPHS)).astype(NP_BF16).copy()

    # x^T per core in slot order (h0 upper feature half zero, built on device).
    xT = np.zeros((N_CORES, IN_DIM, NLP), NP_BF16)
    xTsrc = x.T.astype(NP_BF16)  # [128, N_NODES]
    xT[s_core, :, s_loc] = xTsrc.T  # fancy-index: [N_NODES, 128] rows
    # note: xT[c, :, loc] assignment via advanced indexing transposes; the
    # line above sets xT[core, :, loc] = x[node] for each node.

    wz = weight.reshape(L, 2, P, C).astype(NP_BF16)
    wihT = np.ascontiguousarray(w_ih.T).reshape(2, P, G3).astype(NP_BF16)
    whhT = np.ascontiguousarray(w_hh.T).reshape(2, P, G3).astype(NP_BF16)
    bsum = b_ih + b_hh
    bias = np.stack(
        [bsum[0:128], bsum[128:256], bsum[256:384], bsum[384:512],
         b_ih[512:640], b_ih[640:768], b_hh[512:640], b_hh[640:768]], axis=1
    ).astype(np.float32)
    w1T = np.ascontiguousarray(W1.T).reshape(2, P, 128).astype(np.float32)
    w2T = np.ascontiguousarray(W2.T).astype(np.float32)
    b1c = b1.reshape(P, 1).astype(np.float32)
    b2c = b2.reshape(1, 1).astype(np.float32)

    in_maps = []
    for c in range(N_CORES):
        in_maps.append({
            "xT": xT[c],
            "gidx_a": gidx_a[c],
            "gidx_b": gidx_b[c],
            "dstrow": dstrow[c],
            "gidf": gidf[c],
            "invrow": invrow,
            "iota128": iota128,
            "iotag": iotag,
            "wz": wz,
            "wihT": wihT,
            "whhT": whhT,
            "bias": bias,
            "w1T": w1T,
            "b1": b1c,
            "w2T": w2T,
            "b2": b2c,
        })
    return in_maps, chunks


# --------------------------------------------------------------------------
# Device kernel
# --------------------------------------------------------------------------

def build_kernel(chunks, abl: str = '', reps: int = 1, mdt_name: str = 'bf16'):
    chunks_l = chunks
    koff = [0]
    for v in chunks:
        koff.append(koff[-1] + v)
    ktot = koff[-1]
    MDT = {'bf16': BF16, 'fp8': mybir.dt.float8e4}[mdt_name]
    nc = bacc.Bacc(None, num_devices=N_CORES)

    xT_in = nc.dram_tensor("xT", [IN_DIM, NLP], BF16, kind="ExternalInput")
    gidxa_in = nc.dram_tensor("gidx_a", [P, ktot], I32, kind="ExternalInput")
    gidxb_in = nc.dram_tensor("gidx_b", [P, ktot], I32, kind="ExternalInput")
    dstrow_in = nc.dram_tensor("dstrow", [P, ktot], mybir.dt.uint8,
                               kind="ExternalInput")
    gidf_in = nc.dram_tensor("gidf", [TPC, P], BF16, kind="ExternalInput")
    invrow_in = nc.dram_tensor("invrow", [P, N_GRAPHS], BF16, kind="ExternalInput")
    iota128_in = nc.dram_tensor("iota128", [P, P], BF16, kind="ExternalInput")
    iotag_in = nc.dram_tensor("iotag", [P, N_GRAPHS], BF16, kind="ExternalInput")
    wz_in = nc.dram_tensor("wz", [L, 2, P, C], BF16, kind="ExternalInput")
    wihT_in = nc.dram_tensor("wihT", [2, P, G3], BF16, kind="ExternalInput")
    whhT_in = nc.dram_tensor("whhT", [2, P, G3], BF16, kind="ExternalInput")
    bias_in = nc.dram_tensor("bias", [P, 8], F32, kind="ExternalInput")
    w1T_in = nc.dram_tensor("w1T", [2, P, 128], F32, kind="ExternalInput")
    b1_in = nc.dram_tensor("b1", [P, 1], F32, kind="ExternalInput")
    w2T_in = nc.dram_tensor("w2T", [P, 1], F32, kind="ExternalInput")
    b2_in = nc.dram_tensor("b2", [1, 1], F32, kind="ExternalInput")
    out = nc.dram_tensor("out", [1, N_GRAPHS], F32, kind="ExternalOutput")

    rg = [list(range(N_CORES))]
    NL = L * reps

    with tile.TileContext(nc) as tc:
        with (
            tc.tile_pool(name="persist", bufs=1) as pp,
            tc.tile_pool(name="msb", bufs=1) as msb_pool,
            tc.tile_pool(name="mg", bufs=8) as mg_pool,
            tc.tile_pool(name="ssb", bufs=3) as ssb_pool,
            tc.tile_pool(name="gsb", bufs=10) as gsb_pool,
            tc.tile_pool(name="ps", bufs=6, space="PSUM") as ps_pool,
            tc.tile_pool(name="pps", bufs=1, space="PSUM") as pps_pool,
            tc.tile_pool(name="dram", bufs=1, space="DRAM") as dr,
        ):
            m_loc = dr.tile([NLP, C], MDT, name="m_loc")
            m_fulls_a = [
                dr.tile([NA, C], MDT, addr_space="Shared", name=f"m_full_a{l}")
                for l in range(NL)
            ]
            m_fulls_b = [
                dr.tile([NB, C], MDT, addr_space="Shared", name=f"m_full_b{l}")
                for l in range(NL)
            ]
            sums_loc = dr.tile([C, N_GRAPHS], F32, name="sums_loc")
            sums_full = dr.tile([C, N_GRAPHS], F32, addr_space="Shared",
                                name="sums_full")

            # ---------------- persistent SBUF state ----------------
            h_t = [[None] * len(BLOCKS) for _ in range(2)]
            agg_t = [[None] * len(BLOCKS) for _ in range(2)]
            for cch in range(2):
                for b, (off, nb) in enumerate(BLOCKS):
                    h_t[cch][b] = pp.tile([P, nb], BF16, name=f"h_{cch}_{b}")
                    if cch == 0:
                        nc.sync.dma_start(
                            out=h_t[cch][b][:],
                            in_=xT_in[:, off : off + nb],
                        )
                    else:
                        nc.vector.memset(h_t[cch][b][:], 0.0)
                    agg_t[cch][b] = pp.tile([P, nb], BF16, name=f"agg_{cch}_{b}")

            w_sb = pp.tile([P, L * 2 * C], BF16, name="w_sb")
            for l in range(L):
                for cch in range(2):
                    nc.sync.dma_start(
                        out=w_sb[:, (l * 2 + cch) * C : (l * 2 + cch + 1) * C],
                        in_=wz_in[l, cch],
                    )
            wih_sb = pp.tile([P, 2 * G3], BF16, name="wih_sb")
            whh_sb = pp.tile([P, 2 * G3], BF16, name="whh_sb")
            for cch in range(2):
                nc.sync.dma_start(out=wih_sb[:, cch * G3 : (cch + 1) * G3],
                                  in_=wihT_in[cch])
                nc.sync.dma_start(out=whh_sb[:, cch * G3 : (cch + 1) * G3],
                                  in_=whhT_in[cch])
            bias_sb = pp.tile([P, 8], F32, name="bias_sb")
            nc.sync.dma_start(out=bias_sb[:], in_=bias_in[:])
            w1_sb = pp.tile([P, 2 * 128], F32, name="w1_sb")
            for cch in range(2):
                nc.sync.dma_start(out=w1_sb[:, cch * 128 : (cch + 1) * 128],
                                  in_=w1T_in[cch])
            b1_sb = pp.tile([P, 1], F32, name="b1_sb")
            nc.sync.dma_start(out=b1_sb[:], in_=b1_in[:])
            w2_sb = pp.tile([P, 1], F32, name="w2_sb")
            nc.sync.dma_start(out=w2_sb[:], in_=w2T_in[:])
            b2_sb = pp.tile([1, 1], F32, name="b2_sb")
            nc.sync.dma_start(out=b2_sb[:], in_=b2_in[:])

            zero_sb = pp.tile([P, max(chunks_l) * C], BF16, name="zero_sb")
            nc.vector.memset(zero_sb[:], 0.0)
            gidxa_sb = pp.tile([P, ktot], I32, name="gidxa_sb")
            nc.sync.dma_start(out=gidxa_sb[:], in_=gidxa_in[:])
            gidxb_sb = pp.tile([P, ktot], I32, name="gidxb_sb")
            nc.sync.dma_start(out=gidxb_sb[:], in_=gidxb_in[:])
            ident_sb = pp.tile([P, P], BF16, name="ident_sb")
            make_identity(nc, ident_sb[:])

            dstu8_sb = pp.tile([P, ktot], mybir.dt.uint8, name="dstu8_sb")
            nc.sync.dma_start(out=dstu8_sb[:], in_=dstrow_in[:])
            dstf_sb = pp.tile([P, ktot], BF16, name="dstf_sb")
            nc.vector.tensor_copy(dstf_sb[:], dstu8_sb[:])
            s_all = pp.tile([P, ktot * P], mybir.dt.float8e4, name="s_all")
            gidf_sb = pp.tile([P, TPC], BF16, name="gidf_sb")
            nc.sync.dma_start(
                out=gidf_sb[:],
                in_=gidf_in.rearrange("t p -> p t"),
            )
            invrow_sb = pp.tile([P, N_GRAPHS], BF16, name="invrow_sb")
            nc.sync.dma_start(out=invrow_sb[:], in_=invrow_in[:])
            iota128_sb = pp.tile([P, P], BF16, name="iota128_sb")
            nc.sync.dma_start(out=iota128_sb[:], in_=iota128_in[:])
            iotag_sb = pp.tile([P, N_GRAPHS], BF16, name="iotag_sb")
            nc.sync.dma_start(out=iotag_sb[:], in_=iotag_in[:])

            # node-major message staging for the two AllGather pieces
            m_blk_a = msb_pool.tile([P, TSPA * C], MDT, name="m_blk_a")
            m_blk_b = msb_pool.tile([P, TSPB * C], MDT, name="m_blk_b")

            def wih(cch, gc):
                return wih_sb[:, cch * G3 + gc * P : cch * G3 + (gc + 1) * P]

            def whh(cch, gc):
                return whh_sb[:, cch * G3 + gc * P : cch * G3 + (gc + 1) * P]

            def h_slice(cch, t):
                b = t // 4
                o = (t % 4) * P
                return h_t[cch][b][:, o : o + P]

            def a_tile(l, t):
                """Compute m tile t for layer l into the staging buffer."""
                psm = ps_pool.tile([P, C], F32, name="psm", tag="ps")
                for cch in range(2):
                    nc.tensor.matmul(
                        psm[:],
                        lhsT=h_slice(cch, t),
                        rhs=w_sb[:, (l * 2 + cch) * C : (l * 2 + cch + 1) * C],
                        start=(cch == 0),
                        stop=(cch == 1),
                    )
                if t < TSPA:
                    nc.scalar.copy(m_blk_a[:, t * C : (t + 1) * C], psm[:])
                else:
                    nc.scalar.copy(m_blk_b[:, (t - TSPA) * C : (t - TSPA + 1) * C],
                                   psm[:])

            def store_piece(piece):
                if piece == 0:
                    nc.sync.dma_start(
                        out=m_loc[0:ROWS_A, :].rearrange("(p t) f -> p (t f)", p=P),
                        in_=m_blk_a[:],
                    )
                else:
                    nc.sync.dma_start(
                        out=m_loc[ROWS_A:NLP, :].rearrange("(p t) f -> p (t f)", p=P),
                        in_=m_blk_b[:],
                    )

            def allgather(ll, piece):
                if "cc0" in abl:
                    return
                if "cc1" in abl and ll > 0:
                    return
                li = 0 if "cc1" in abl else ll
                if piece == 0:
                    nc.gpsimd.collective_compute(
                        "AllGather",
                        mybir.AluOpType.bypass,
                        replica_groups=rg,
                        ins=[m_loc[0:ROWS_A, :]],
                        outs=[m_fulls_a[li][:]],
                    )
                else:
                    nc.gpsimd.collective_compute(
                        "AllGather",
                        mybir.AluOpType.bypass,
                        replica_groups=rg,
                        ins=[m_loc[ROWS_A:NLP, :]],
                        outs=[m_fulls_b[li][:]],
                    )

            def gru_block(b):
                off, nb = BLOCKS[b]
                z_keep = [None, None]
                n_keep = [None, None]
                for gh in range(2):
                    psr = ps_pool.tile([P, nb], F32, name="psr", tag="ps")
                    nc.tensor.matmul(psr[:], lhsT=wih(0, gh), rhs=agg_t[0][b][:],
                                     start=True, stop=False)
                    nc.tensor.matmul(psr[:], lhsT=wih(1, gh), rhs=agg_t[1][b][:],
                                     start=False, stop=False)
                    nc.tensor.matmul(psr[:], lhsT=whh(0, gh), rhs=h_t[0][b][:],
                                     start=False, stop=False)
                    nc.tensor.matmul(psr[:], lhsT=whh(1, gh), rhs=h_t[1][b][:],
                                     start=False, stop=True)
                    r_sb = gsb_pool.tile([P, nb], BF16, name="r_sb", tag="gate")
                    nc.scalar.activation(
                        r_sb[:], psr[:], mybir.ActivationFunctionType.Sigmoid,
                        bias=bias_sb[:, gh : gh + 1],
                    )
                    psz = ps_pool.tile([P, nb], F32, name="psz", tag="ps")
                    nc.tensor.matmul(psz[:], lhsT=wih(0, 2 + gh),
                                     rhs=agg_t[0][b][:], start=True, stop=False)
                    nc.tensor.matmul(psz[:], lhsT=wih(1, 2 + gh),
                                     rhs=agg_t[1][b][:], start=False, stop=False)
                    nc.tensor.matmul(psz[:], lhsT=whh(0, 2 + gh),
                                     rhs=h_t[0][b][:], start=False, stop=False)
                    nc.tensor.matmul(psz[:], lhsT=whh(1, 2 + gh),
                                     rhs=h_t[1][b][:], start=False, stop=True)
                    z_sb = gsb_pool.tile([P, nb], BF16, name="z_sb", tag="gate")
                    nc.scalar.activation(
                        z_sb[:], psz[:], mybir.ActivationFunctionType.Sigmoid,
                        bias=bias_sb[:, 2 + gh : 3 + gh],
                    )
                    psi = ps_pool.tile([P, nb], F32, name="psi", tag="ps")
                    nc.tensor.matmul(psi[:], lhsT=wih(0, 4 + gh),
                                     rhs=agg_t[0][b][:], start=True, stop=False)
                    nc.tensor.matmul(psi[:], lhsT=wih(1, 4 + gh),
                                     rhs=agg_t[1][b][:], start=False, stop=True)
                    psh = ps_pool.tile([P, nb], F32, name="psh", tag="ps")
                    nc.tensor.matmul(psh[:], lhsT=whh(0, 4 + gh),
                                     rhs=h_t[0][b][:], start=True, stop=False)
                    nc.tensor.matmul(psh[:], lhsT=whh(1, 4 + gh),
                                     rhs=h_t[1][b][:], start=False, stop=True)
                    hn_sb = gsb_pool.tile([P, nb], BF16, name="hn_sb", tag="gate")
                    nc.scalar.activation(
                        hn_sb[:], psh[:], mybir.ActivationFunctionType.Identity,
                        bias=bias_sb[:, 6 + gh : 7 + gh],
                    )
                    rn_sb = gsb_pool.tile([P, nb], BF16, name="rn_sb", tag="gate")
                    nc.vector.tensor_mul(rn_sb[:], r_sb[:], hn_sb[:])
                    tn_sb = gsb_pool.tile([P, nb], F32, name="tn_sb", tag="gate")
                    nc.vector.tensor_add(tn_sb[:], psi[:], rn_sb[:])
                    n_sb = gsb_pool.tile([P, nb], BF16, name="n_sb", tag="gate")
                    nc.scalar.activation(
                        n_sb[:], tn_sb[:], mybir.ActivationFunctionType.Tanh,
                        bias=bias_sb[:, 4 + gh : 5 + gh],
                    )
                    z_keep[gh] = z_sb
                    n_keep[gh] = n_sb
                for gh in range(2):
                    d_sb = gsb_pool.tile([P, nb], BF16, name="d_sb", tag="gate")
                    nc.vector.tensor_sub(d_sb[:], h_t[gh][b][:], n_keep[gh][:])
                    zd_sb = gsb_pool.tile([P, nb], BF16, name="zd_sb", tag="gate")
                    nc.vector.tensor_mul(zd_sb[:], z_keep[gh][:], d_sb[:])
                    nc.vector.tensor_add(h_t[gh][b][:], n_keep[gh][:], zd_sb[:])

            # Mean-pool accumulators + per-tile pooling work; interleaved
            # into the LAST layer's per-block slot (where earlier layers
            # compute the next layer's m tiles) so it hides under the final
            # gather drain.
            pp0 = pps_pool.tile([P, N_GRAPHS], F32, name="pp0")
            pp1 = pps_pool.tile([P, N_GRAPHS], F32, name="pp1")
            ppx = [pp0, pp1]

            def pool_tile(t):
                h_rm = ssb_pool.tile([P, C], BF16, name="h_rm")
                for cch in range(2):
                    ptr = ps_pool.tile([P, P], BF16, name="ptr", tag="ps")
                    nc.tensor.transpose(ptr[:], h_slice(cch, t), ident_sb[:])
                    nc.vector.tensor_copy(h_rm[:, cch * P : (cch + 1) * P], ptr[:])
                b_sb = ssb_pool.tile([P, N_GRAPHS], BF16, name="b_sb")
                nc.vector.tensor_tensor(
                    out=b_sb[:],
                    in0=iotag_sb[:],
                    in1=gidf_sb[:, t : t + 1].to_broadcast([P, N_GRAPHS]),
                    op=mybir.AluOpType.is_equal,
                )
                nc.vector.tensor_mul(b_sb[:], b_sb[:], invrow_sb[:])
                for fh in range(2):
                    nc.tensor.matmul(
                        ppx[fh][:],
                        lhsT=h_rm[:, fh * P : (fh + 1) * P],
                        rhs=b_sb[:],
                        start=(t == 0),
                        stop=(t == TPC - 1),
                    )

            # ---------------- layer 0 prologue: m(0) and its AllGather ------
            for t in range(TPC):
                a_tile(0, t)
                if t == TSPA - 1:
                    store_piece(0)
                    allgather(0, 0)
            store_piece(1)
            allgather(0, 1)

            # ---------------- layers ----------------
            DLEAD = 0  # A-gather lead (0: HW A/B showed a lead delays the critical B-stream) (tiles): drains table-A chunks while
                       # the table-B AllGather is still in flight

            for ll in range(NL):
                l = ll % L
                li = 0 if "cc1" in abl else ll
                m_full_a = m_fulls_a[li]
                m_full_b = m_fulls_b[li]

                def issue_a(t):
                    """Memset + issue the table-A gathers for dst tile t.

                    Table-A gathers only wait on m_full_a (AllGathered mid
                    previous layer), so with a DLEAD-tile lead they drain
                    while this layer's table-B AllGather completes."""
                    ks, ke = koff[t], koff[t + 1]
                    nk = ke - ks
                    m_gA = mg_pool.tile([P, nk * C], MDT, name="m_gA", tag="mga", bufs=8)
                    nc.scalar.copy(m_gA[:], zero_sb[:, : nk * C])
                    ng = 0 if "g0" in abl else (1 if "g1" in abl else nk)
                    for k in range(ng):
                        nc.gpsimd.indirect_dma_start(
                            out=m_gA[:, k * C : (k + 1) * C],
                            out_offset=None,
                            in_=m_full_a[:],
                            in_offset=bass.IndirectOffsetOnAxis(
                                ap=gidxa_sb[:, ks + k : ks + k + 1],
                                axis=0,
                            ),
                            bounds_check=NA - 1,
                            oob_is_err=False,
                        )
                    return m_gA

                ag_a = {}
                for t in range(min(DLEAD, TPC)):
                    ag_a[t] = issue_a(t)

                # Phase C interleaved with GRU and next layer's m tiles.
                for t in range(TPC):
                    ks, ke = koff[t], koff[t + 1]
                    nk = ke - ks
                    if t + DLEAD < TPC:
                        ag_a[t + DLEAD] = issue_a(t + DLEAD)
                    m_gA = ag_a.pop(t)
                    m_gB = mg_pool.tile([P, nk * C], MDT, name="m_gB", tag="mgb", bufs=4)
                    nc.scalar.copy(m_gB[:], zero_sb[:, : nk * C])
                    ng = 0 if "g0" in abl else (1 if "g1" in abl else nk)
                    for k in range(ng):
                        nc.gpsimd.indirect_dma_start(
                            out=m_gB[:, k * C : (k + 1) * C],
                            out_offset=None,
                            in_=m_full_b[:],
                            in_offset=bass.IndirectOffsetOnAxis(
                                ap=gidxb_sb[:, ks + k : ks + k + 1],
                                axis=0,
                            ),
                            bounds_check=NB - 1,
                            oob_is_err=False,
                        )
                    if ll == 0:
                        for k in range(nk):
                            nc.vector.tensor_tensor(
                                out=s_all[:, (ks + k) * P : (ks + k + 1) * P],
                                in0=iota128_sb[:],
                                in1=dstf_sb[
                                    :, ks + k : ks + k + 1
                                ].to_broadcast([P, P]),
                                op=mybir.AluOpType.is_equal,
                            )
                    bq = t // 4
                    oq = (t % 4) * P
                    for fh in range(2):
                        psa = ps_pool.tile([P, P], F32, name="psa", tag="ps")
                        for k in range(nk):
                            nc.tensor.matmul(
                                psa[:],
                                lhsT=m_gA[:, k * C + fh * P : k * C + fh * P + P],
                                rhs=s_all[:, (ks + k) * P : (ks + k + 1) * P],
                                start=(k == 0),
                                stop=False,
                            )
                            nc.tensor.matmul(
                                psa[:],
                                lhsT=m_gB[:, k * C + fh * P : k * C + fh * P + P],
                                rhs=s_all[:, (ks + k) * P : (ks + k + 1) * P],
                                start=False,
                                stop=(k == nk - 1),
                            )
                        nc.vector.tensor_copy(
                            agg_t[fh][bq][:, oq : oq + P], psa[:]
                        )
                    # When a 4-tile block completes: GRU it, then compute the
                    # next layer's m tiles for it (overlaps the gather drain).
                    last_of_block = (t % 4 == 3) or (t == TPC - 1)
                    if last_of_block and "nogru" not in abl:
                        b = t // 4
                        gru_block(b)
                        if ll < NL - 1:
                            for tq in range(b * 4, min(b * 4 + 4, TPC)):
                                a_tile((ll + 1) % L, tq)
                            if t == TSPA - 1:
                                store_piece(0)
                            elif t == 31:
                                # traced 8 tiles after the piece-A store so the
                                # Pool queue reaches this trigger with its wait
                                # already satisfied (no gather bubble)
                                allgather(ll + 1, 0)
                            elif t == TPC - 1:
                                store_piece(1)
                                allgather(ll + 1, 1)
                        else:
                            for tq in range(b * 4, min(b * 4 + 4, TPC)):
                                pool_tile(tq)

            # ---------------- pooling tail ----------------
            if "nogru" in abl:
                for t in range(TPC):
                    pool_tile(t)
            sums_sb = pp.tile([P, 2 * N_GRAPHS], F32, name="sums_sb")
            nc.scalar.copy(sums_sb[:, 0:N_GRAPHS], pp0[:])
            nc.scalar.copy(sums_sb[:, N_GRAPHS : 2 * N_GRAPHS], pp1[:])
            nc.sync.dma_start(out=sums_loc[0:P, :], in_=sums_sb[:, 0:N_GRAPHS])
            nc.sync.dma_start(out=sums_loc[P : 2 * P, :],
                              in_=sums_sb[:, N_GRAPHS : 2 * N_GRAPHS])

            nc.gpsimd.collective_compute(
                "AllReduce",
                mybir.AluOpType.add,
                replica_groups=rg,
                ins=[sums_loc[:]],
                outs=[sums_full[:]],
            )

            # ---------------- classifier ----------------
            mt_sb = pp.tile([P, 2 * N_GRAPHS], F32, name="mt_sb")
            nc.sync.dma_start(out=mt_sb[:, 0:N_GRAPHS], in_=sums_full[0:P, :])
            nc.sync.dma_start(out=mt_sb[:, N_GRAPHS : 2 * N_GRAPHS],
                              in_=sums_full[P : 2 * P, :])
            ps1 = ps_pool.tile([P, N_GRAPHS], F32, name="ps1", tag="ps")
            for cch in range(2):
                nc.tensor.matmul(
                    ps1[:],
                    lhsT=w1_sb[:, cch * 128 : (cch + 1) * 128],
                    rhs=mt_sb[:, cch * N_GRAPHS : (cch + 1) * N_GRAPHS],
                    start=(cch == 0),
                    stop=(cch == 1),
                )
            h1_sb = pp.tile([P, N_GRAPHS], F32, name="h1_sb")
            nc.scalar.activation(
                h1_sb[:], ps1[:], mybir.ActivationFunctionType.Relu,
                bias=b1_sb[:, 0:1],
            )
            ps2 = ps_pool.tile([1, N_GRAPHS], F32, name="ps2", tag="ps")
            nc.tensor.matmul(ps2[:], lhsT=w2_sb[:, 0:1], rhs=h1_sb[:],
                             start=True, stop=True)
            out_sb = pp.tile([1, N_GRAPHS], F32, name="out_sb")
            nc.scalar.activation(
                out_sb[:], ps2[:], mybir.ActivationFunctionType.Sigmoid,
                bias=b2_sb[:, 0:1],
            )
            nc.sync.dma_start(out=out[:], in_=out_sb[:])

    nc.finalize()
    return nc


# --------------------------------------------------------------------------
# PJRT SPMD runner
# --------------------------------------------------------------------------

class SpmdRunner:
    def __init__(self, nc, n_cores):
        import jax
        from jax.experimental.shard_map import shard_map
        from jax.sharding import Mesh, PartitionSpec
        from concourse.bass2jax import (
            _bass_exec_p,
            install_neuronx_cc_hook,
            partition_id_tensor,
        )

        install_neuronx_cc_hook()
        self.jax = jax
        self.nc = nc
        self.n_cores = n_cores

        partition_name = nc.partition_id_tensor.name if nc.partition_id_tensor else None
        in_names, out_names, out_avals, zero_outs = [], [], [], []
        for alloc in nc.m.functions[0].allocations:
            if not isinstance(alloc, mybir.MemoryLocationSet):
                continue
            if not alloc.memorylocations:
                continue
            name = alloc.memorylocations[0].name
            if alloc.kind == "ExternalInput":
                if name != partition_name:
                    in_names.append(name)
            elif alloc.kind == "ExternalOutput":
                shape = tuple(alloc.tensor_shape)
                dtype = mybir.dt.np(alloc.dtype)
                out_names.append(name)
                out_avals.append(jax.core.ShapedArray(shape, dtype))
                zero_outs.append(np.zeros(shape, dtype))
        self.in_names = in_names
        self.out_names = out_names
        self.out_avals = out_avals
        self.zero_outs = zero_outs
        n_params = len(in_names)
        n_outs = len(out_names)
        all_in_names = list(in_names) + list(out_names)
        if partition_name is not None:
            all_in_names.append(partition_name)

        def _body(*args):
            operands = list(args)
            if partition_name is not None:
                operands.append(partition_id_tensor())
            outs = _bass_exec_p.bind(
                *operands,
                out_avals=tuple(out_avals),
                in_names=tuple(all_in_names),
                out_names=tuple(out_names),
                lowering_input_output_aliases=(),
                sim_require_finite=True,
                sim_require_nnan=True,
                nc=nc,
            )
            return tuple(outs)

        devices = jax.devices()[:n_cores]
        assert len(devices) == n_cores, (
            f"need {n_cores} neuron cores, found {len(jax.devices())}"
        )
        mesh = Mesh(np.asarray(devices), ("core",))
        in_specs = (PartitionSpec("core"),) * (n_params + n_outs)
        out_specs = (PartitionSpec("core"),) * n_outs
        self.fn = jax.jit(
            shard_map(_body, mesh=mesh, in_specs=in_specs, out_specs=out_specs,
                      check_rep=False),
            keep_unused=True,
        )

    def prepare(self, in_maps):
        n = self.n_cores
        concat_in = [
            np.concatenate([np.asarray(in_maps[c][name]) for c in range(n)], axis=0)
            for name in self.in_names
        ]
        concat_zero = [
            np.zeros((n * z.shape[0], *z.shape[1:]), z.dtype) for z in self.zero_outs
        ]
        return [self.jax.device_put(a) for a in concat_in + concat_zero]

    def run(self, dev_args):
        outs = self.fn(*dev_args)
        self.jax.block_until_ready(outs)
        return outs

    def results(self, outs):
        n = self.n_cores
        return [
            {
                name: np.asarray(outs[i]).reshape(n, *self.out_avals[i].shape)[c]
                for i, name in enumerate(self.out_names)
            }
            for c in range(n)
        ]


_RUNNER_CACHE = {}


def get_runner(chunks) -> SpmdRunner:
    if chunks not in _RUNNER_CACHE:
        _RUNNER_CACHE[chunks] = SpmdRunner(build_kernel(chunks), N_CORES)
    return _RUNNER_CACHE[chunks]


def kernel(**inputs) -> np.ndarray:
    in_maps, chunks = preprocess(**inputs)
    runner = get_runner(chunks)
    dev_args = runner.prepare(in_maps)
    outs = runner.run(dev_args)
    res = runner.results(outs)
    probs = np.asarray(res[0]["out"], np.float32).reshape(1, N_GRAPHS)
    return probs.T.copy()
